# revision 1
# baseline (speedup 1.0000x reference)
"""Trainium2 Bass kernel for CSNetModel GNN message passing (8 NeuronCores).

Strategy: shard destination nodes across the 8 cores (12500 each). Each layer's
segment_sum is computed with one-hot matmuls on the tensor engine over
host-sorted edge chunks; per-edge features are fetched with indirect DMA
gathers from replicated (AllGather'd) bf16 node tables. Feature transforms are
fused before aggregation (GCN: gather pre-transformed tables) or after
(RGCN/Hetero: per-relation PSUM banks + weight matmuls). All index arithmetic
is done on the host; the device program is identical across cores (SPMD), with
per-core edge data padded to a uniform chunk/segment schedule.
"""
import math
import numpy as np
import ml_dtypes

import jax
from jax.sharding import Mesh, PartitionSpec, NamedSharding
from jax.experimental.shard_map import shard_map

import concourse.bass as bass
import concourse.bacc as bacc
import concourse.tile as tile
import concourse.mybir as mybir
from concourse.bass2jax import _bass_exec_p, install_neuronx_cc_hook, partition_id_tensor

F32 = mybir.dt.float32
BF16 = mybir.dt.bfloat16
I32 = mybir.dt.int32

NCORES = 8
N = 100000
NLOC = N // NCORES          # 12500
D = 128
NTILE = (NLOC + 127) // 128  # 98
LAST_W = NLOC - (NTILE - 1) * 128  # 84
R_HET = 4
R_RG = 8

TDT = BF16                   # table / matmul dtype
TNP = ml_dtypes.bfloat16

ALIGN = {"gcn1": False, "gcn2": False, "rg1": False, "rg2": False,
         "het1": False, "het2": False}


def _tw(t):
    return 128 if t < NTILE - 1 else LAST_W


# ---------------------------------------------------------------------------
# Host-side edge packing
# ---------------------------------------------------------------------------

def pack_layer(src, dst, rel, R, align):
    """Build SPMD-uniform chunk/segment schedule for one layer-graph.

    src, dst: int arrays [E] (global node ids); rel: int array [E] or None.
    Returns dict with nchunk, nseg, groups (ordered list), and per-core
    idx_mat [128, nchunk] int32 / dl_mat [128, nseg] float32.
    """
    src = np.asarray(src).astype(np.int64)
    dst = np.asarray(dst).astype(np.int64)
    rel = np.zeros_like(src) if rel is None else np.asarray(rel).astype(np.int64)
    core = dst // NLOC
    dl = dst % NLOC
    tl = dl // 128
    dloc = dl % 128
    g = tl * R + rel
    NG = NTILE * R

    counts = np.zeros((NCORES, NG), np.int64)
    percore = []
    for c in range(NCORES):
        m = core == c
        gc = g[m]
        order = np.argsort(gc, kind="stable")
        gc = gc[order]
        percore.append((gc, src[m][order], dloc[m][order]))
        counts[c] = np.bincount(gc, minlength=NG)
    NE = counts.max(axis=0)

    if align:
        sizes = ((NE + 127) // 128) * 128
    else:
        sizes = NE.copy()
    off = np.zeros(NG + 1, np.int64)
    np.cumsum(sizes, out=off[1:])
    total = int(off[-1])
    nchunk = (total + 127) // 128
    tot_pad = nchunk * 128

    groups = []
    nseg = 0
    for gi in range(NG):
        ne = int(NE[gi])
        if ne == 0:
            continue
        lo, hi = int(off[gi]), int(off[gi]) + ne
        segs = []
        for k in range(lo // 128, (hi - 1) // 128 + 1):
            segs.append((k, nseg))
            nseg += 1
        groups.append({"t": gi // R, "r": gi % R, "lo": lo, "hi": hi, "segs": segs})

    group_of = {(grp["t"], grp["r"]): grp for grp in groups}

    idx_mats, dl_mats = [], []
    starts = off[:-1]
    for c in range(NCORES):
        gc, srcs, dlocs = percore[c]
        first_occ = np.searchsorted(gc, np.arange(NG))
        pos = starts[gc] + (np.arange(len(gc)) - first_occ[gc])
        idx_flat = np.zeros(tot_pad, np.int32)
        idx_flat[pos] = srcs
        dl_flat = np.full(tot_pad, -1.0, np.float32)
        dl_flat[pos] = dlocs
        idx_mats.append(np.ascontiguousarray(idx_flat.reshape(nchunk, 128).T))
        dl_mat = np.full((128, max(nseg, 1)), -1.0, np.float32)
        for grp in groups:
            for (k, col) in grp["segs"]:
                s = max(grp["lo"], k * 128)
                e = min(grp["hi"], (k + 1) * 128)
                colv = np.full(128, -1.0, np.float32)
                colv[s - k * 128:e - k * 128] = dl_flat[s:e]
                dl_mat[:, col] = colv
        dl_mats.append(dl_mat)

    return {"nchunk": nchunk, "nseg": max(nseg, 1), "groups": groups,
            "group_of": group_of, "idx": idx_mats, "dl": dl_mats}


# ---------------------------------------------------------------------------
# Device program
# ---------------------------------------------------------------------------

def build_program(plans, stages=("prep", "ag01", "l1", "ag234", "l2"),
                  loop_r=None):
    stages = set(stages)
    noag = "noag" in stages
    nc = bacc.Bacc("TRN2", target_bir_lowering=False, debug=False,
                   num_devices=NCORES)

    # --- external inputs (per core) ---
    ext = {}

    def din(name, shape, dt):
        ext[name] = nc.dram_tensor(name, list(shape), dt, kind="ExternalInput")
        return ext[name]

    emb_sl = din("emb_sl", [NLOC, D], TDT)
    gcn_W1 = din("gcn_W1", [D, D], TDT)
    gcn_W2 = din("gcn_W2", [D, D], TDT)
    gcn_b1 = din("gcn_b1", [D, 1], F32)
    gcn_b2r = din("gcn_b2r", [D, D], F32)          # row-broadcast bias
    rg_W1 = din("rg_W1", [D, R_RG * D], TDT)
    rg_W2 = din("rg_W2", [D, R_RG * D], TDT)
    rg_loop1 = din("rg_loop1", [D, D], TDT)
    rg_loop2 = din("rg_loop2", [D, D], TDT)
    rg_b1 = din("rg_b1", [D, 1], F32)
    rg_b2 = din("rg_b2", [D, 1], F32)
    het_W1 = din("het_W1", [D, R_HET * D], TDT)
    het_W2 = din("het_W2", [D, R_HET * D], TDT)    # pre-scaled by 0.25 on host
    het_b1 = din("het_b1", [D, R_HET], F32)
    het_b2 = din("het_b2", [D, R_HET], F32)
    iota_in = din("iota", [D, D], TDT)
    ident_b = din("ident_b", [D, D], TDT)
    ident_f = din("ident_f", [D, D], F32)
    for lname in ("gcn1", "gcn2", "rg1", "rg2", "het1", "het2"):
        p = plans[lname]
        din(f"idx_{lname}", [128, p["nchunk"]], I32)
        din(f"dl_{lname}", [128, p["nseg"]], TDT)

    hcf_out = nc.dram_tensor("hcf", [NLOC, D], F32, kind="ExternalOutput")
    hc_out = nc.dram_tensor("hc", [NLOC, D], F32, kind="ExternalOutput")
    hs_out = nc.dram_tensor("hs", [NLOC, D], F32, kind="ExternalOutput")

    if noag:
        ext_tabs = {}
        for nm in ("emb_full_in", "t1_full_in", "t2_full_in", "h1_full_in",
                   "hs1_full_in"):
            ext_tabs[nm] = nc.dram_tensor(nm, [N, D], TDT, kind="ExternalInput")

    Tanh = mybir.ActivationFunctionType.Tanh
    AG = "AllGather"
    RGROUPS = [list(range(NCORES))]

    with tile.TileContext(nc) as tc:
        with tc.tile_pool(name="consts", bufs=1) as cp, \
             tc.tile_pool(name="gat", bufs=28) as gp, \
             tc.tile_pool(name="oh", bufs=12) as ohp, \
             tc.tile_pool(name="work", bufs=6) as wp, \
             tc.tile_pool(name="psb", bufs=3, space="PSUM") as psb, \
             tc.tile_pool(name="pss", bufs=3, space="PSUM") as pss, \
             tc.tile_pool(name="ptr", bufs=2, space="PSUM") as ptr, \
             tc.tile_pool(name="dram", bufs=1, space="DRAM") as dp:

            # --- constant tiles ---
            def load_const(name, shape, dt):
                t = cp.tile(list(shape), dt, tag=name)
                nc.sync.dma_start(out=t[:], in_=ext[name][:])
                return t

            iota_t = load_const("iota", [D, D], TDT)
            identb_t = load_const("ident_b", [D, D], TDT)
            identf_t = load_const("ident_f", [D, D], F32)
            gW1_t = load_const("gcn_W1", [D, D], TDT)
            gW2_t = load_const("gcn_W2", [D, D], TDT)
            gb1_t = load_const("gcn_b1", [D, 1], F32)
            gb2r_t = load_const("gcn_b2r", [D, D], F32)
            rW1_t = load_const("rg_W1", [D, R_RG * D], TDT)
            rW2_t = load_const("rg_W2", [D, R_RG * D], TDT)
            rL1_t = load_const("rg_loop1", [D, D], TDT)
            rL2_t = load_const("rg_loop2", [D, D], TDT)
            rb1_t = load_const("rg_b1", [D, 1], F32)
            rb2_t = load_const("rg_b2", [D, 1], F32)
            hW1_t = load_const("het_W1", [D, R_HET * D], TDT)
            hW2_t = load_const("het_W2", [D, R_HET * D], TDT)
            hb1_t = load_const("het_b1", [D, R_HET], F32)
            hb2_t = load_const("het_b2", [D, R_HET], F32)
            meta = {}
            for lname in ("gcn1", "gcn2", "rg1", "rg2", "het1", "het2"):
                p = plans[lname]
                meta[lname] = (
                    load_const(f"idx_{lname}", [128, p["nchunk"]], I32),
                    load_const(f"dl_{lname}", [128, p["nseg"]], TDT),
                )

            # --- internal DRAM ---
            emb_bounce = dp.tile([NLOC, D], TDT, tag="emb_b")
            emb_full = dp.tile([N, D], TDT, tag="emb_f", addr_space="Shared")
            loop_ctx = (tc.For_i(0, loop_r, 1)
                        if (loop_r and "agrep" not in stages) else None)
            if loop_ctx:
                loop_ctx.__enter__()
            t1_bounce = dp.tile([NLOC, D], TDT, tag="t1_b")
            t1_full = dp.tile([N, D], TDT, tag="t1_f", addr_space="Shared")
            t2_bounce = dp.tile([NLOC, D], TDT, tag="t2_b")
            t2_full = dp.tile([N, D], TDT, tag="t2_f", addr_space="Shared")
            h1_bounce = dp.tile([NLOC, D], TDT, tag="h1_b")
            h1_full = dp.tile([N, D], TDT, tag="h1_f", addr_space="Shared")
            hs1_bounce = dp.tile([NLOC, D], TDT, tag="hs1_b")
            hs1_full = dp.tile([N, D], TDT, tag="hs1_f", addr_space="Shared")
            embT_dram = dp.tile([D, NLOC], TDT, tag="embT")
            h1T_dram = dp.tile([D, NLOC], TDT, tag="h1T")

            # --- prep: embT tiles, T1 = emb @ W1, bounces ---
            nc.sync.dma_start(out=emb_bounce[:], in_=emb_sl[:])
            for t in range(NTILE):
                w = _tw(t)
                e_sb = wp.tile([128, D], TDT, tag="embt")
                if w < 128:
                    nc.vector.memset(e_sb[:], 0.0)
                nc.sync.dma_start(out=e_sb[:w, :], in_=emb_sl[t * 128:t * 128 + w, :])
                trp = ptr.tile([128, D], TDT, tag="ptr")
                nc.tensor.transpose(out=trp[:], in_=e_sb[:], identity=identb_t[:])
                eT = wp.tile([128, D], TDT, tag="eT")
                nc.vector.tensor_copy(out=eT[:], in_=trp[:])
                nc.sync.dma_start(out=embT_dram[:, t * 128:t * 128 + w],
                                  in_=eT[:, :w])
                t1p = pss.tile([128, D], F32, tag="pss")
                nc.tensor.matmul(out=t1p[:], lhsT=eT[:], rhs=gW1_t[:],
                                 start=True, stop=True)
                t1sb = wp.tile([128, D], TDT, tag="t1sb")
                nc.vector.tensor_copy(out=t1sb[:], in_=t1p[:])
                nc.sync.dma_start(out=t1_bounce[t * 128:t * 128 + w, :],
                                  in_=t1sb[:w, :])

            agrep = loop_r if (loop_r and "agrep" in stages) else 1
            if "ag01" in stages:
                for _ in range(agrep):
                    nc.gpsimd.collective_compute(
                        AG, mybir.AluOpType.bypass, replica_groups=RGROUPS,
                        ins=[emb_bounce.opt()], outs=[emb_full.opt()])
                    nc.gpsimd.collective_compute(
                        AG, mybir.AluOpType.bypass, replica_groups=RGROUPS,
                        ins=[t1_bounce.opt()], outs=[t1_full.opt()])

            # --- shared layer machinery ---
            def gather_fn(lname, table):
                idx_t, _ = meta[lname]
                cache = {}

                def gather(k):
                    if k not in cache:
                        gt = gp.tile([128, D], TDT, tag="gat")
                        nc.gpsimd.indirect_dma_start(
                            out=gt[:], out_offset=None, in_=table[:],
                            in_offset=bass.IndirectOffsetOnAxis(
                                ap=idx_t[:, k:k + 1], axis=0))
                        cache[k] = gt
                    return cache[k]
                return gather

            def onehot(lname, col):
                _, dl_t = meta[lname]
                oh = ohp.tile([128, D], TDT, tag="oh")
                nc.vector.tensor_tensor(
                    out=oh[:], in0=dl_t[:, col:col + 1].to_broadcast([128, D]),
                    in1=iota_t[:], op=mybir.AluOpType.is_equal)
                return oh

            def accum_group(lname, gather, grp, bank, bcol, transposed):
                """Accumulate one (tile, rel) group into bank[:, bcol:bcol+128].

                transposed=True -> out[f, d] (lhsT=msgs, rhs=onehot)
                transposed=False -> out[d, f] (lhsT=onehot, rhs=msgs)
                """
                segs = grp["segs"]
                for si, (k, col) in enumerate(segs):
                    gt = gather(k)
                    oh = onehot(lname, col)
                    lhsT, rhs = (gt, oh) if transposed else (oh, gt)
                    nc.tensor.matmul(out=bank[:, bcol:bcol + 128],
                                     lhsT=lhsT[:], rhs=rhs[:],
                                     start=(si == 0), stop=(si == len(segs) - 1))

            # =========== GCN layer 1 (gathers T1; aggT supertiles) ==========
            def emit_gcn1():
                lname = "gcn1"
                plan = plans[lname]
                gather = gather_fn(lname, t1_full)
                for st in range((NTILE + 3) // 4):
                    tls = list(range(st * 4, min(st * 4 + 4, NTILE)))
                    bank = psb.tile([128, 512], F32, tag="psb")
                    for j, t in enumerate(tls):
                        grp = plan["group_of"].get((t, 0))
                        if grp is None:
                            nc.vector.memset(bank[:, j * 128:(j + 1) * 128], 0.0)
                            continue
                        accum_group(lname, gather, grp, bank, j * 128, True)
                    w = 128 * len(tls)
                    h1T = wp.tile([128, 512], TDT, tag="h1Tst")
                    nc.scalar.activation(h1T[:, :w], bank[:, :w], Tanh,
                                         bias=gb1_t[:], scale=1.0)
                    for j, t in enumerate(tls):
                        tp = pss.tile([128, D], F32, tag="pss")
                        nc.tensor.matmul(out=tp[:],
                                         lhsT=h1T[:, j * 128:(j + 1) * 128],
                                         rhs=gW2_t[:], start=True, stop=True)
                        tsb = wp.tile([128, D], TDT, tag="t2sb")
                        nc.vector.tensor_copy(out=tsb[:], in_=tp[:])
                        nc.sync.dma_start(
                            out=t2_bounce[t * 128:t * 128 + _tw(t), :],
                            in_=tsb[:_tw(t), :])

            # =========== GCN layer 2 (gathers T2; agg per tile) =============
            def emit_gcn2():
                lname = "gcn2"
                plan = plans[lname]
                gather = gather_fn(lname, t2_full)
                for t in range(NTILE):
                    grp = plan["group_of"].get((t, 0))
                    pt = pss.tile([128, D], F32, tag="pss")
                    if grp is None:
                        nc.vector.memset(pt[:], 0.0)
                    else:
                        accum_group(lname, gather, grp, pt, 0, False)
                    tmp = wp.tile([128, D], F32, tag="g2tmp")
                    nc.vector.tensor_add(out=tmp[:], in0=pt[:], in1=gb2r_t[:])
                    ot = wp.tile([128, D], F32, tag="g2out")
                    nc.scalar.activation(ot[:], tmp[:], Tanh)
                    nc.sync.dma_start(out=hcf_out[t * 128:t * 128 + _tw(t), :],
                                      in_=ot[:_tw(t), :])

            # =========== RGCN layer (B banks per rel + transforms) ==========
            def emit_rg(lname, table, xT_src, W_t, loop_t, b_t, first):
                plan = plans[lname]
                gather = gather_fn(lname, table)
                for t in range(NTILE):
                    w = _tw(t)
                    quads = []
                    for qi in range(2):
                        q = psb.tile([128, 512], F32, tag="psb")
                        quads.append(q)
                    for r in range(R_RG):
                        grp = plan["group_of"].get((t, r))
                        q, qc = quads[r // 4], (r % 4) * 128
                        if grp is None:
                            nc.vector.memset(q[:, qc:qc + 128], 0.0)
                        else:
                            accum_group(lname, gather, grp, q, qc, True)
                    stages = []
                    for qi in range(2):
                        s = wp.tile([128, 512], TDT, tag="stage")
                        nc.vector.tensor_copy(out=s[:], in_=quads[qi][:])
                        stages.append(s)
                    xT_t = wp.tile([128, D], TDT, tag="xTt")
                    nc.sync.dma_start(out=xT_t[:, :w],
                                      in_=xT_src[:, t * 128:t * 128 + w])
                    ot = pss.tile([128, D], F32, tag="pss")
                    nc.tensor.matmul(out=ot[:], lhsT=loop_t[:], rhs=xT_t[:],
                                     start=True, stop=False)
                    for r in range(R_RG):
                        nc.tensor.matmul(
                            out=ot[:], lhsT=W_t[:, r * 128:(r + 1) * 128],
                            rhs=stages[r // 4][:, (r % 4) * 128:(r % 4 + 1) * 128],
                            start=False, stop=(r == R_RG - 1))
                    if first:
                        hT = wp.tile([128, D], TDT, tag="hTb")
                        nc.scalar.activation(hT[:], ot[:], Tanh, bias=b_t[:],
                                             scale=1.0)
                        nc.sync.dma_start(
                            out=h1T_dram[:, t * 128:t * 128 + w], in_=hT[:, :w])
                        trp = ptr.tile([128, D], TDT, tag="ptr")
                        nc.tensor.transpose(out=trp[:], in_=hT[:],
                                            identity=identb_t[:])
                        hsb = wp.tile([128, D], TDT, tag="hsbb")
                        nc.vector.tensor_copy(out=hsb[:], in_=trp[:])
                        nc.sync.dma_start(out=h1_bounce[t * 128:t * 128 + w, :],
                                          in_=hsb[:w, :])
                    else:
                        hTf = wp.tile([128, D], F32, tag="hTf")
                        nc.scalar.activation(hTf[:], ot[:], Tanh, bias=b_t[:],
                                             scale=1.0)
                        trp = ptr.tile([128, D], F32, tag="ptr")
                        nc.tensor.transpose(out=trp[:], in_=hTf[:],
                                            identity=identf_t[:])
                        hsb = wp.tile([128, D], F32, tag="hsbf")
                        nc.vector.tensor_copy(out=hsb[:], in_=trp[:])
                        nc.sync.dma_start(out=hc_out[t * 128:t * 128 + w, :],
                                          in_=hsb[:w, :])

            # =========== Hetero layer (4 rels, mean of tanh) ================
            def emit_het(lname, table, W_t, b_t, first):
                plan = plans[lname]
                gather = gather_fn(lname, table)
                for t in range(NTILE):
                    w = _tw(t)
                    quad = psb.tile([128, 512], F32, tag="psb")
                    for r in range(R_HET):
                        grp = plan["group_of"].get((t, r))
                        if grp is None:
                            nc.vector.memset(quad[:, r * 128:(r + 1) * 128], 0.0)
                        else:
                            accum_group(lname, gather, grp, quad, r * 128, True)
                    stage = wp.tile([128, 512], TDT, tag="stage")
                    nc.vector.tensor_copy(out=stage[:], in_=quad[:])
                    acc = wp.tile([128, D], F32, tag="hacc")
                    for r in range(R_HET):
                        otr = pss.tile([128, D], F32, tag="pss")
                        nc.tensor.matmul(
                            out=otr[:], lhsT=W_t[:, r * 128:(r + 1) * 128],
                            rhs=stage[:, r * 128:(r + 1) * 128],
                            start=True, stop=True)
                        if r == 0:
                            nc.scalar.activation(acc[:], otr[:], Tanh,
                                                 bias=b_t[:, 0:1], scale=1.0)
                        else:
                            tmp = wp.tile([128, D], F32, tag="htmp")
                            nc.scalar.activation(tmp[:], otr[:], Tanh,
                                                 bias=b_t[:, r:r + 1], scale=1.0)
                            nc.vector.tensor_add(out=acc[:], in0=acc[:],
                                                 in1=tmp[:])
                    if first:
                        # no 0.25 scale: folded into het_W2 on host
                        hsT = wp.tile([128, D], TDT, tag="hTb")
                        nc.vector.tensor_copy(out=hsT[:], in_=acc[:])
                        trp = ptr.tile([128, D], TDT, tag="ptr")
                        nc.tensor.transpose(out=trp[:], in_=hsT[:],
                                            identity=identb_t[:])
                        hsb = wp.tile([128, D], TDT, tag="hsbb")
                        nc.vector.tensor_copy(out=hsb[:], in_=trp[:])
                        nc.sync.dma_start(out=hs1_bounce[t * 128:t * 128 + w, :],
                                          in_=hsb[:w, :])
                    else:
                        hsT = wp.tile([128, D], F32, tag="hTf")
                        nc.vector.tensor_scalar_mul(hsT[:], acc[:], 0.25)
                        trp = ptr.tile([128, D], F32, tag="ptr")
                        nc.tensor.transpose(out=trp[:], in_=hsT[:],
                                            identity=identf_t[:])
                        hsb = wp.tile([128, D], F32, tag="hsbf")
                        nc.vector.tensor_copy(out=hsb[:], in_=trp[:])
                        nc.sync.dma_start(out=hs_out[t * 128:t * 128 + w, :],
                                          in_=hsb[:w, :])

            # --- emit layers ---
            if noag:
                emb_full = ext_tabs["emb_full_in"]
                t1_full = ext_tabs["t1_full_in"]
                t2_full = ext_tabs["t2_full_in"]
                h1_full = ext_tabs["h1_full_in"]
                hs1_full = ext_tabs["hs1_full_in"]
            if "l1" in stages or "l1rg" in stages:
                emit_rg("rg1", emb_full, embT_dram, rW1_t, rL1_t, rb1_t, True)
            if "l1" in stages or "l1het" in stages:
                emit_het("het1", emb_full, hW1_t, hb1_t, True)
            if "l1" in stages or "l1gcn" in stages:
                emit_gcn1()

            if "ag234" in stages:
                for _ in range(agrep):
                    nc.gpsimd.collective_compute(
                        AG, mybir.AluOpType.bypass, replica_groups=RGROUPS,
                        ins=[h1_bounce.opt()], outs=[h1_full.opt()])
                    nc.gpsimd.collective_compute(
                        AG, mybir.AluOpType.bypass, replica_groups=RGROUPS,
                        ins=[hs1_bounce.opt()], outs=[hs1_full.opt()])
                    nc.gpsimd.collective_compute(
                        AG, mybir.AluOpType.bypass, replica_groups=RGROUPS,
                        ins=[t2_bounce.opt()], outs=[t2_full.opt()])

            if "l2" in stages or "l2rg" in stages:
                emit_rg("rg2", h1_full, h1T_dram, rW2_t, rL2_t, rb2_t, False)
            if "l2" in stages or "l2het" in stages:
                emit_het("het2", hs1_full, hW2_t, hb2_t, False)
            if "l2" in stages or "l2gcn" in stages:
                emit_gcn2()
            if loop_ctx:
                loop_ctx.__exit__(None, None, None)

    nc.compile()
    return nc


# ---------------------------------------------------------------------------
# Runner (PJRT via axon)
# ---------------------------------------------------------------------------

class _Runner:
    def __init__(self, nc, n_cores):
        install_neuronx_cc_hook()
        self.n_cores = n_cores
        partition_name = (nc.partition_id_tensor.name
                          if nc.partition_id_tensor else None)
        in_names, out_names, out_avals, zero_outs = [], [], [], []
        for alloc in nc.m.functions[0].allocations:
            if not isinstance(alloc, mybir.MemoryLocationSet):
                continue
            name = alloc.memorylocations[0].name
            if alloc.kind == "ExternalInput":
                if name != partition_name:
                    in_names.append(name)
            elif alloc.kind == "ExternalOutput":
                shape = tuple(alloc.tensor_shape)
                dtype = mybir.dt.np(alloc.dtype)
                out_avals.append(jax.core.ShapedArray(shape, dtype))
                out_names.append(name)
                zero_outs.append(np.zeros(shape, dtype))
        self.in_names, self.out_names = in_names, out_names
        self.out_avals, self.zero_outs = out_avals, zero_outs
        n_params, n_outs = len(in_names), len(out_avals)
        all_in = list(in_names) + list(out_names)
        if partition_name is not None:
            all_in.append(partition_name)

        def _body(*args):
            operands = list(args)
            if partition_name is not None:
                operands.append(partition_id_tensor())
            return tuple(_bass_exec_p.bind(
                *operands, out_avals=tuple(out_avals), in_names=tuple(all_in),
                out_names=tuple(out_names), lowering_input_output_aliases=(),
                sim_require_finite=True, sim_require_nnan=True, nc=nc))

        devices = jax.devices()[:n_cores]
        self.mesh = Mesh(np.asarray(devices), ("core",))
        in_specs = (PartitionSpec("core"),) * (n_params + n_outs)
        out_specs = (PartitionSpec("core"),) * n_outs
        donate = tuple(range(n_params, n_params + n_outs))
        self.fn = jax.jit(
            shard_map(_body, mesh=self.mesh, in_specs=in_specs,
                      out_specs=out_specs, check_rep=False),
            donate_argnums=donate, keep_unused=True)
        self.sharding = NamedSharding(self.mesh, PartitionSpec("core"))

    def put_inputs(self, in_maps):
        n = self.n_cores
        per_core = [[np.asarray(m[k]) for k in self.in_names] for m in in_maps]
        self.dev_in = [
            jax.device_put(
                np.concatenate([per_core[c][i] for c in range(n)], axis=0),
                self.sharding)
            for i in range(len(self.in_names))
        ]
        for a in self.dev_in:
            a.block_until_ready()

    def _make_zeros(self):
        if not hasattr(self, "_zfn"):
            n = self.n_cores
            shapes = [((n * z.shape[0],) + z.shape[1:], z.dtype)
                      for z in self.zero_outs]
            import jax.numpy as jnp

            def zf():
                return tuple(jnp.zeros(s, d) for s, d in shapes)
            self._zfn = jax.jit(zf, out_shardings=tuple(
                [self.sharding] * len(shapes)))
        zs = self._zfn()
        for z in zs:
            z.block_until_ready()
        return list(zs)

    def run(self, fetch=True):
        n = self.n_cores
        zs = self._make_zeros()
        outs = self.fn(*self.dev_in, *zs)
        for o in outs:
            o.block_until_ready()
        if not fetch:
            return None
        return [
            {name: np.asarray(outs[i]).reshape(n, *self.out_avals[i].shape)[c]
             for i, name in enumerate(self.out_names)}
            for c in range(n)
        ]


# ---------------------------------------------------------------------------
# Entry point
# ---------------------------------------------------------------------------

_LAST_RUNNER = None


def kernel(gcn_src1, gcn_dst1, gcn_src2, gcn_dst2,
           rg_src1, rg_dst1, rg_et1, rg_src2, rg_dst2, rg_et2,
           het_src1, het_dst1, het_src2, het_dst2,
           emb, gcn_W1, gcn_b1, gcn_W2, gcn_b2,
           rg_W1, rg_loop1, rg_b1, rg_W2, rg_loop2, rg_b2,
           het_W1, het_b1, het_W2, het_b2):
    emb = np.asarray(emb, np.float32)

    # hetero edge lists: concatenate the 4 relations with rel tags
    def het_edges(srcs, dsts):
        s = np.concatenate([np.asarray(srcs[r]).ravel() for r in range(R_HET)])
        d = np.concatenate([np.asarray(dsts[r]).ravel() for r in range(R_HET)])
        r = np.concatenate([np.full(np.asarray(srcs[r]).size, r, np.int64)
                            for r in range(R_HET)])
        return s, d, r

    hs1_, hd1_, hr1_ = het_edges(het_src1, het_dst1)
    hs2_, hd2_, hr2_ = het_edges(het_src2, het_dst2)

    plans = {
        "gcn1": pack_layer(gcn_src1, gcn_dst1, None, 1, ALIGN["gcn1"]),
        "gcn2": pack_layer(gcn_src2, gcn_dst2, None, 1, ALIGN["gcn2"]),
        "rg1": pack_layer(rg_src1, rg_dst1, rg_et1, R_RG, ALIGN["rg1"]),
        "rg2": pack_layer(rg_src2, rg_dst2, rg_et2, R_RG, ALIGN["rg2"]),
        "het1": pack_layer(hs1_, hd1_, hr1_, R_HET, ALIGN["het1"]),
        "het2": pack_layer(hs2_, hd2_, hr2_, R_HET, ALIGN["het2"]),
    }

    nc = build_program(plans)
    runner = _Runner(nc, NCORES)

    iota_np = np.broadcast_to(np.arange(D, dtype=np.float32), (D, D))
    shared = {
        "gcn_W1": np.asarray(gcn_W1).astype(TNP),
        "gcn_W2": np.asarray(gcn_W2).astype(TNP),
        "gcn_b1": np.asarray(gcn_b1, np.float32).reshape(D, 1),
        "gcn_b2r": np.broadcast_to(np.asarray(gcn_b2, np.float32), (D, D)).copy(),
        "rg_W1": np.concatenate([np.asarray(rg_W1)[r] for r in range(R_RG)],
                                axis=1).astype(TNP),
        "rg_W2": np.concatenate([np.asarray(rg_W2)[r] for r in range(R_RG)],
                                axis=1).astype(TNP),
        "rg_loop1": np.asarray(rg_loop1).astype(TNP),
        "rg_loop2": np.asarray(rg_loop2).astype(TNP),
        "rg_b1": np.asarray(rg_b1, np.float32).reshape(D, 1),
        "rg_b2": np.asarray(rg_b2, np.float32).reshape(D, 1),
        "het_W1": np.concatenate([np.asarray(het_W1)[r] for r in range(R_HET)],
                                 axis=1).astype(TNP),
        "het_W2": np.concatenate([0.25 * np.asarray(het_W2)[r]
                                  for r in range(R_HET)], axis=1).astype(TNP),
        "het_b1": np.ascontiguousarray(np.asarray(het_b1, np.float32).T),
        "het_b2": np.ascontiguousarray(np.asarray(het_b2, np.float32).T),
        "iota": iota_np.astype(TNP),
        "ident_b": np.eye(D, dtype=TNP),
        "ident_f": np.eye(D, dtype=np.float32),
    }

    in_maps = []
    for c in range(NCORES):
        m = dict(shared)
        m["emb_sl"] = emb[c * NLOC:(c + 1) * NLOC, :].astype(TNP)
        for lname in ("gcn1", "gcn2", "rg1", "rg2", "het1", "het2"):
            m[f"idx_{lname}"] = plans[lname]["idx"][c]
            m[f"dl_{lname}"] = plans[lname]["dl"][c].astype(TNP)
        in_maps.append(m)

    global _LAST_RUNNER
    _LAST_RUNNER = runner
    runner.put_inputs(in_maps)
    res = runner.run()

    hcf = np.concatenate([res[c]["hcf"] for c in range(NCORES)], axis=0)
    hc = np.concatenate([res[c]["hc"] for c in range(NCORES)], axis=0)
    hs = np.concatenate([res[c]["hs"] for c in range(NCORES)], axis=0)
    return (hcf, hc, hs)



# revision 7
# speedup vs baseline: 1.9657x; 1.9657x over previous
"""Trainium2 Bass kernel for CSNetModel GNN message passing (8 NeuronCores).

Strategy: shard destination nodes across the 8 cores (12500 each). Each layer's
segment_sum is computed with one-hot matmuls on the tensor engine over
host-sorted edge chunks; per-edge features are fetched with indirect DMA
gathers from replicated (AllGather'd) bf16 node tables. Feature transforms are
fused before aggregation (GCN: gather pre-transformed tables) or after
(RGCN/Hetero: per-relation PSUM banks + weight matmuls). All index arithmetic
is done on the host; the device program is identical across cores (SPMD), with
per-core edge data padded to a uniform chunk/segment schedule.

I/O layout note: through this runtime path, every ExternalInput/Output costs
per-exec marshalling proportional to its number of DRAM rows (dim0), so all
external tensors are shaped [128, *] (wide). Outputs are produced transposed
([feat, node]) and un-transposed on the host; emb arrives tile-transposed.
"""
import math
import numpy as np
import ml_dtypes

import jax
from jax.sharding import Mesh, PartitionSpec, NamedSharding
from jax.experimental.shard_map import shard_map

import concourse.bass as bass
import concourse.bacc as bacc
import concourse.tile as tile
import concourse.mybir as mybir
from concourse.bass2jax import _bass_exec_p, install_neuronx_cc_hook, partition_id_tensor

F32 = mybir.dt.float32
BF16 = mybir.dt.bfloat16
I32 = mybir.dt.int32

NCORES = 8
N = 100000
NLOC = N // NCORES          # 12500
D = 128
NTILE = (NLOC + 127) // 128  # 98
LAST_W = NLOC - (NTILE - 1) * 128  # 84
R_HET = 4
R_RG = 8

TDT = BF16                   # table / matmul dtype
TNP = ml_dtypes.bfloat16

ALIGN = {"gcn1": False, "gcn2": False, "rg1": False, "rg2": False,
         "het1": False, "het2": False}


def _tw(t):
    return 128 if t < NTILE - 1 else LAST_W


# ---------------------------------------------------------------------------
# Host-side edge packing
# ---------------------------------------------------------------------------

def pack_layer(src, dst, rel, R, align):
    """Build SPMD-uniform chunk/segment schedule for one layer-graph.

    src, dst: int arrays [E] (global node ids); rel: int array [E] or None.
    Returns dict with nchunk, nseg, groups (ordered list), and per-core
    idx_mat [128, nchunk] int32 / dl_mat [128, nseg] float32.
    """
    src = np.asarray(src).astype(np.int64)
    dst = np.asarray(dst).astype(np.int64)
    rel = np.zeros_like(src) if rel is None else np.asarray(rel).astype(np.int64)
    core = dst // NLOC
    dl = dst % NLOC
    tl = dl // 128
    dloc = dl % 128
    g = tl * R + rel
    NG = NTILE * R

    counts = np.zeros((NCORES, NG), np.int64)
    percore = []
    for c in range(NCORES):
        m = core == c
        gc = g[m]
        order = np.argsort(gc, kind="stable")
        gc = gc[order]
        percore.append((gc, src[m][order], dloc[m][order]))
        counts[c] = np.bincount(gc, minlength=NG)
    NE = counts.max(axis=0)

    if align:
        sizes = ((NE + 127) // 128) * 128
    else:
        sizes = NE.copy()
    off = np.zeros(NG + 1, np.int64)
    np.cumsum(sizes, out=off[1:])
    total = int(off[-1])
    nchunk = (total + 127) // 128
    tot_pad = nchunk * 128

    groups = []
    nseg = 0
    for gi in range(NG):
        ne = int(NE[gi])
        if ne == 0:
            continue
        lo, hi = int(off[gi]), int(off[gi]) + ne
        segs = []
        for k in range(lo // 128, (hi - 1) // 128 + 1):
            segs.append((k, nseg))
            nseg += 1
        groups.append({"t": gi // R, "r": gi % R, "lo": lo, "hi": hi, "segs": segs})

    group_of = {(grp["t"], grp["r"]): grp for grp in groups}

    idx_mats, dl_mats = [], []
    starts = off[:-1]
    for c in range(NCORES):
        gc, srcs, dlocs = percore[c]
        first_occ = np.searchsorted(gc, np.arange(NG))
        pos = starts[gc] + (np.arange(len(gc)) - first_occ[gc])
        idx_flat = np.zeros(tot_pad, np.int32)
        idx_flat[pos] = srcs
        dl_flat = np.full(tot_pad, -1.0, np.float32)
        dl_flat[pos] = dlocs
        idx_mats.append(np.ascontiguousarray(idx_flat.reshape(nchunk, 128).T))
        dl_mat = np.full((128, max(nseg, 1)), -1.0, np.float32)
        for grp in groups:
            for (k, col) in grp["segs"]:
                s = max(grp["lo"], k * 128)
                e = min(grp["hi"], (k + 1) * 128)
                colv = np.full(128, -1.0, np.float32)
                colv[s - k * 128:e - k * 128] = dl_flat[s:e]
                dl_mat[:, col] = colv
        dl_mats.append(dl_mat)

    return {"nchunk": nchunk, "nseg": max(nseg, 1), "groups": groups,
            "group_of": group_of, "idx": idx_mats, "dl": dl_mats}


# ---------------------------------------------------------------------------
# Device program
# ---------------------------------------------------------------------------

def build_program(plans, stages=("prep", "ag01", "l1", "ag234", "l2")):
    stages = set(stages)
    noag = "noag" in stages
    nc = bacc.Bacc("TRN2", target_bir_lowering=False, debug=False,
                   num_devices=NCORES)

    # --- external inputs (per core) ---
    ext = {}

    def din(name, shape, dt):
        ext[name] = nc.dram_tensor(name, list(shape), dt, kind="ExternalInput")
        return ext[name]

    embT_in = din("embT_in", [D, NTILE * 128], TDT)  # tile-transposed emb slice
    gcn_W1 = din("gcn_W1", [D, D], TDT)
    gcn_W2 = din("gcn_W2", [D, D], TDT)
    gcn_b1 = din("gcn_b1", [D, 1], F32)
    gcn_b2 = din("gcn_b2", [D, 1], F32)
    rg_W1 = din("rg_W1", [D, R_RG * D], TDT)
    rg_W2 = din("rg_W2", [D, R_RG * D], TDT)
    rg_loop1 = din("rg_loop1", [D, D], TDT)
    rg_loop2 = din("rg_loop2", [D, D], TDT)
    rg_b1 = din("rg_b1", [D, 1], F32)
    rg_b2 = din("rg_b2", [D, 1], F32)
    het_W1 = din("het_W1", [D, R_HET * D], TDT)
    het_W2 = din("het_W2", [D, R_HET * D], TDT)    # pre-scaled by 0.25 on host
    het_b1 = din("het_b1", [D, R_HET], F32)
    het_b2 = din("het_b2", [D, R_HET], F32)
    iota_in = din("iota", [D, D], TDT)
    ident_b = din("ident_b", [D, D], TDT)
    for lname in ("gcn1", "gcn2", "rg1", "rg2", "het1", "het2"):
        p = plans[lname]
        din(f"idx_{lname}", [128, p["nchunk"]], I32)
        din(f"dl_{lname}", [128, p["nseg"]], TDT)

    # transposed outputs: [feat, node] — wide layout is cheap to marshal
    hcf_out = nc.dram_tensor("hcfT", [D, NLOC], F32, kind="ExternalOutput")
    hc_out = nc.dram_tensor("hcT", [D, NLOC], F32, kind="ExternalOutput")
    hs_out = nc.dram_tensor("hsT", [D, NLOC], F32, kind="ExternalOutput")

    if noag:
        ext_tabs = {}
        for nm in ("emb_full_in", "t1_full_in", "t2_full_in", "h1_full_in",
                   "hs1_full_in"):
            ext_tabs[nm] = nc.dram_tensor(nm, [N, D], TDT, kind="ExternalInput")

    Tanh = mybir.ActivationFunctionType.Tanh
    AG = "AllGather"
    RGROUPS = [list(range(NCORES))]

    with tile.TileContext(nc) as tc:
        with tc.tile_pool(name="consts", bufs=1) as cp, \
             tc.tile_pool(name="gat", bufs=28) as gp, \
             tc.tile_pool(name="oh", bufs=12) as ohp, \
             tc.tile_pool(name="work", bufs=6) as wp, \
             tc.tile_pool(name="psb", bufs=3, space="PSUM") as psb, \
             tc.tile_pool(name="pss", bufs=3, space="PSUM") as pss, \
             tc.tile_pool(name="ptr", bufs=2, space="PSUM") as ptr, \
             tc.tile_pool(name="dram", bufs=1, space="DRAM") as dp:

            # --- constant tiles ---
            def load_const(name, shape, dt):
                t = cp.tile(list(shape), dt, tag=name)
                nc.sync.dma_start(out=t[:], in_=ext[name][:])
                return t

            iota_t = load_const("iota", [D, D], TDT)
            identb_t = load_const("ident_b", [D, D], TDT)
            gW1_t = load_const("gcn_W1", [D, D], TDT)
            gW2_t = load_const("gcn_W2", [D, D], TDT)
            gb1_t = load_const("gcn_b1", [D, 1], F32)
            gb2_t = load_const("gcn_b2", [D, 1], F32)
            rW1_t = load_const("rg_W1", [D, R_RG * D], TDT)
            rW2_t = load_const("rg_W2", [D, R_RG * D], TDT)
            rL1_t = load_const("rg_loop1", [D, D], TDT)
            rL2_t = load_const("rg_loop2", [D, D], TDT)
            rb1_t = load_const("rg_b1", [D, 1], F32)
            rb2_t = load_const("rg_b2", [D, 1], F32)
            hW1_t = load_const("het_W1", [D, R_HET * D], TDT)
            hW2_t = load_const("het_W2", [D, R_HET * D], TDT)
            hb1_t = load_const("het_b1", [D, R_HET], F32)
            hb2_t = load_const("het_b2", [D, R_HET], F32)
            meta = {}
            for lname in ("gcn1", "gcn2", "rg1", "rg2", "het1", "het2"):
                p = plans[lname]
                meta[lname] = (
                    load_const(f"idx_{lname}", [128, p["nchunk"]], I32),
                    load_const(f"dl_{lname}", [128, p["nseg"]], TDT),
                )

            # --- internal DRAM ---
            emb_bounce = dp.tile([NLOC, D], TDT, tag="emb_b")
            emb_full = dp.tile([N, D], TDT, tag="emb_f", addr_space="Shared")
            t1_bounce = dp.tile([NLOC, D], TDT, tag="t1_b")
            t1_full = dp.tile([N, D], TDT, tag="t1_f", addr_space="Shared")
            t2_bounce = dp.tile([NLOC, D], TDT, tag="t2_b")
            t2_full = dp.tile([N, D], TDT, tag="t2_f", addr_space="Shared")
            h1_bounce = dp.tile([NLOC, D], TDT, tag="h1_b")
            h1_full = dp.tile([N, D], TDT, tag="h1_f", addr_space="Shared")
            hs1_bounce = dp.tile([NLOC, D], TDT, tag="hs1_b")
            hs1_full = dp.tile([N, D], TDT, tag="hs1_f", addr_space="Shared")
            h1T_dram = dp.tile([D, NLOC], TDT, tag="h1T")

            # --- prep: emb_bounce rows from embT tiles, T1 = emb @ W1 ---
            for t in range(NTILE):
                w = _tw(t)
                eT = wp.tile([128, D], TDT, tag="eT")
                nc.sync.dma_start(out=eT[:], in_=embT_in[:, t * 128:t * 128 + 128])
                trp = ptr.tile([128, D], TDT, tag="ptr")
                nc.tensor.transpose(out=trp[:], in_=eT[:], identity=identb_t[:])
                e_sb = wp.tile([128, D], TDT, tag="embt")
                nc.vector.tensor_copy(out=e_sb[:], in_=trp[:])
                nc.sync.dma_start(out=emb_bounce[t * 128:t * 128 + w, :],
                                  in_=e_sb[:w, :])
                t1p = pss.tile([128, D], F32, tag="pss")
                nc.tensor.matmul(out=t1p[:], lhsT=eT[:], rhs=gW1_t[:],
                                 start=True, stop=True)
                t1sb = wp.tile([128, D], TDT, tag="t1sb")
                nc.vector.tensor_copy(out=t1sb[:], in_=t1p[:])
                nc.sync.dma_start(out=t1_bounce[t * 128:t * 128 + w, :],
                                  in_=t1sb[:w, :])

            if "ag01" in stages:
                nc.gpsimd.collective_compute(
                    AG, mybir.AluOpType.bypass, replica_groups=RGROUPS,
                    ins=[emb_bounce.opt()], outs=[emb_full.opt()])
                nc.gpsimd.collective_compute(
                    AG, mybir.AluOpType.bypass, replica_groups=RGROUPS,
                    ins=[t1_bounce.opt()], outs=[t1_full.opt()])

            # --- shared layer machinery ---
            def gather_fn(lname, table):
                idx_t, _ = meta[lname]
                cache = {}

                def gather(k):
                    if k not in cache:
                        gt = gp.tile([128, D], TDT, tag="gat")
                        nc.gpsimd.indirect_dma_start(
                            out=gt[:], out_offset=None, in_=table[:],
                            in_offset=bass.IndirectOffsetOnAxis(
                                ap=idx_t[:, k:k + 1], axis=0))
                        cache[k] = gt
                    return cache[k]
                return gather

            def onehot(lname, col):
                _, dl_t = meta[lname]
                oh = ohp.tile([128, D], TDT, tag="oh")
                nc.vector.tensor_tensor(
                    out=oh[:], in0=dl_t[:, col:col + 1].to_broadcast([128, D]),
                    in1=iota_t[:], op=mybir.AluOpType.is_equal)
                return oh

            def accum_group(lname, gather, grp, bank, bcol, transposed):
                """Accumulate one (tile, rel) group into bank[:, bcol:bcol+128].

                transposed=True -> out[f, d] (lhsT=msgs, rhs=onehot)
                transposed=False -> out[d, f] (lhsT=onehot, rhs=msgs)
                """
                segs = grp["segs"]
                for si, (k, col) in enumerate(segs):
                    gt = gather(k)
                    oh = onehot(lname, col)
                    lhsT, rhs = (gt, oh) if transposed else (oh, gt)
                    nc.tensor.matmul(out=bank[:, bcol:bcol + 128],
                                     lhsT=lhsT[:], rhs=rhs[:],
                                     start=(si == 0), stop=(si == len(segs) - 1))

            # =========== GCN layer 1 (gathers T1; aggT supertiles) ==========
            def emit_gcn1():
                lname = "gcn1"
                plan = plans[lname]
                gather = gather_fn(lname, t1_full)
                for st in range((NTILE + 3) // 4):
                    tls = list(range(st * 4, min(st * 4 + 4, NTILE)))
                    bank = psb.tile([128, 512], F32, tag="psb")
                    for j, t in enumerate(tls):
                        grp = plan["group_of"].get((t, 0))
                        if grp is None:
                            nc.vector.memset(bank[:, j * 128:(j + 1) * 128], 0.0)
                            continue
                        accum_group(lname, gather, grp, bank, j * 128, True)
                    w = 128 * len(tls)
                    h1T = wp.tile([128, 512], TDT, tag="h1Tst")
                    nc.scalar.activation(h1T[:, :w], bank[:, :w], Tanh,
                                         bias=gb1_t[:], scale=1.0)
                    for j, t in enumerate(tls):
                        tp = pss.tile([128, D], F32, tag="pss")
                        nc.tensor.matmul(out=tp[:],
                                         lhsT=h1T[:, j * 128:(j + 1) * 128],
                                         rhs=gW2_t[:], start=True, stop=True)
                        tsb = wp.tile([128, D], TDT, tag="t2sb")
                        nc.vector.tensor_copy(out=tsb[:], in_=tp[:])
                        nc.sync.dma_start(
                            out=t2_bounce[t * 128:t * 128 + _tw(t), :],
                            in_=tsb[:_tw(t), :])

            # ====== GCN layer 2 (gathers T2; agg transposed, direct out) ====
            def emit_gcn2():
                lname = "gcn2"
                plan = plans[lname]
                gather = gather_fn(lname, t2_full)
                for t in range(NTILE):
                    w = _tw(t)
                    grp = plan["group_of"].get((t, 0))
                    pt = pss.tile([128, D], F32, tag="pss")
                    if grp is None:
                        nc.vector.memset(pt[:], 0.0)
                    else:
                        accum_group(lname, gather, grp, pt, 0, True)
                    ot = wp.tile([128, D], F32, tag="g2out")
                    nc.scalar.activation(ot[:], pt[:], Tanh, bias=gb2_t[:],
                                         scale=1.0)
                    nc.sync.dma_start(out=hcf_out[:, t * 128:t * 128 + w],
                                      in_=ot[:, :w])

            # =========== RGCN layer (B banks per rel + transforms) ==========
            def emit_rg(lname, table, xT_src, W_t, loop_t, b_t, first):
                plan = plans[lname]
                gather = gather_fn(lname, table)
                for t in range(NTILE):
                    w = _tw(t)
                    quads = []
                    for qi in range(2):
                        q = psb.tile([128, 512], F32, tag="psb")
                        quads.append(q)
                    for r in range(R_RG):
                        grp = plan["group_of"].get((t, r))
                        q, qc = quads[r // 4], (r % 4) * 128
                        if grp is None:
                            nc.vector.memset(q[:, qc:qc + 128], 0.0)
                        else:
                            accum_group(lname, gather, grp, q, qc, True)
                    stages_sb = []
                    for qi in range(2):
                        s = wp.tile([128, 512], TDT, tag="stage")
                        nc.vector.tensor_copy(out=s[:], in_=quads[qi][:])
                        stages_sb.append(s)
                    xT_t = wp.tile([128, D], TDT, tag="xTt")
                    nc.sync.dma_start(out=xT_t[:, :w],
                                      in_=xT_src[:, t * 128:t * 128 + w])
                    ot = pss.tile([128, D], F32, tag="pss")
                    nc.tensor.matmul(out=ot[:], lhsT=loop_t[:], rhs=xT_t[:],
                                     start=True, stop=False)
                    for r in range(R_RG):
                        nc.tensor.matmul(
                            out=ot[:], lhsT=W_t[:, r * 128:(r + 1) * 128],
                            rhs=stages_sb[r // 4][:, (r % 4) * 128:(r % 4 + 1) * 128],
                            start=False, stop=(r == R_RG - 1))
                    if first:
                        hT = wp.tile([128, D], TDT, tag="hTb")
                        nc.scalar.activation(hT[:], ot[:], Tanh, bias=b_t[:],
                                             scale=1.0)
                        nc.sync.dma_start(
                            out=h1T_dram[:, t * 128:t * 128 + w], in_=hT[:, :w])
                        trp = ptr.tile([128, D], TDT, tag="ptr")
                        nc.tensor.transpose(out=trp[:], in_=hT[:],
                                            identity=identb_t[:])
                        hsb = wp.tile([128, D], TDT, tag="hsbb")
                        nc.vector.tensor_copy(out=hsb[:], in_=trp[:])
                        nc.sync.dma_start(out=h1_bounce[t * 128:t * 128 + w, :],
                                          in_=hsb[:w, :])
                    else:
                        hTf = wp.tile([128, D], F32, tag="hTf")
                        nc.scalar.activation(hTf[:], ot[:], Tanh, bias=b_t[:],
                                             scale=1.0)
                        nc.sync.dma_start(out=hc_out[:, t * 128:t * 128 + w],
                                          in_=hTf[:, :w])

            # =========== Hetero layer (4 rels, mean of tanh) ================
            def emit_het(lname, table, W_t, b_t, first):
                plan = plans[lname]
                gather = gather_fn(lname, table)
                for t in range(NTILE):
                    w = _tw(t)
                    quad = psb.tile([128, 512], F32, tag="psb")
                    for r in range(R_HET):
                        grp = plan["group_of"].get((t, r))
                        if grp is None:
                            nc.vector.memset(quad[:, r * 128:(r + 1) * 128], 0.0)
                        else:
                            accum_group(lname, gather, grp, quad, r * 128, True)
                    stage = wp.tile([128, 512], TDT, tag="stage")
                    nc.vector.tensor_copy(out=stage[:], in_=quad[:])
                    acc = wp.tile([128, D], F32, tag="hacc")
                    for r in range(R_HET):
                        otr = pss.tile([128, D], F32, tag="pss")
                        nc.tensor.matmul(
                            out=otr[:], lhsT=W_t[:, r * 128:(r + 1) * 128],
                            rhs=stage[:, r * 128:(r + 1) * 128],
                            start=True, stop=True)
                        if r == 0:
                            nc.scalar.activation(acc[:], otr[:], Tanh,
                                                 bias=b_t[:, 0:1], scale=1.0)
                        else:
                            tmp = wp.tile([128, D], F32, tag="htmp")
                            nc.scalar.activation(tmp[:], otr[:], Tanh,
                                                 bias=b_t[:, r:r + 1], scale=1.0)
                            nc.vector.tensor_add(out=acc[:], in0=acc[:],
                                                 in1=tmp[:])
                    if first:
                        # layer-1 mean (0.25) is folded into het_W2 on host
                        hsT = wp.tile([128, D], TDT, tag="hTb")
                        nc.vector.tensor_copy(out=hsT[:], in_=acc[:])
                        trp = ptr.tile([128, D], TDT, tag="ptr")
                        nc.tensor.transpose(out=trp[:], in_=hsT[:],
                                            identity=identb_t[:])
                        hsb = wp.tile([128, D], TDT, tag="hsbb")
                        nc.vector.tensor_copy(out=hsb[:], in_=trp[:])
                        nc.sync.dma_start(out=hs1_bounce[t * 128:t * 128 + w, :],
                                          in_=hsb[:w, :])
                    else:
                        hsT = wp.tile([128, D], F32, tag="hTf")
                        nc.vector.tensor_scalar_mul(hsT[:], acc[:], 0.25)
                        nc.sync.dma_start(out=hs_out[:, t * 128:t * 128 + w],
                                          in_=hsT[:, :w])

            # --- emit layers ---
            emb_tab, t1_tab, t2_tab = emb_full, t1_full, t2_full
            h1_tab, hs1_tab = h1_full, hs1_full
            if noag:
                emb_tab = ext_tabs["emb_full_in"]
                t1_tab = ext_tabs["t1_full_in"]
                t2_tab = ext_tabs["t2_full_in"]
                h1_tab = ext_tabs["h1_full_in"]
                hs1_tab = ext_tabs["hs1_full_in"]
            if "l1" in stages or "l1rg" in stages:
                emit_rg("rg1", emb_tab, embT_in, rW1_t, rL1_t, rb1_t, True)
            if "l1" in stages or "l1het" in stages:
                emit_het("het1", emb_tab, hW1_t, hb1_t, True)
            if "l1" in stages or "l1gcn" in stages:
                emit_gcn1()

            if "ag234" in stages:
                nc.gpsimd.collective_compute(
                    AG, mybir.AluOpType.bypass, replica_groups=RGROUPS,
                    ins=[h1_bounce.opt()], outs=[h1_full.opt()])
                nc.gpsimd.collective_compute(
                    AG, mybir.AluOpType.bypass, replica_groups=RGROUPS,
                    ins=[hs1_bounce.opt()], outs=[hs1_full.opt()])
                nc.gpsimd.collective_compute(
                    AG, mybir.AluOpType.bypass, replica_groups=RGROUPS,
                    ins=[t2_bounce.opt()], outs=[t2_full.opt()])

            if "l2" in stages or "l2rg" in stages:
                emit_rg("rg2", h1_tab, h1T_dram, rW2_t, rL2_t, rb2_t, False)
            if "l2" in stages or "l2het" in stages:
                emit_het("het2", hs1_tab, hW2_t, hb2_t, False)
            if "l2" in stages or "l2gcn" in stages:
                emit_gcn2()

    nc.compile()
    return nc


# ---------------------------------------------------------------------------
# Runner (PJRT via axon)
# ---------------------------------------------------------------------------

class _Runner:
    def __init__(self, nc, n_cores):
        install_neuronx_cc_hook()
        self.n_cores = n_cores
        partition_name = (nc.partition_id_tensor.name
                          if nc.partition_id_tensor else None)
        in_names, out_names, out_avals = [], [], []
        for alloc in nc.m.functions[0].allocations:
            if not isinstance(alloc, mybir.MemoryLocationSet):
                continue
            name = alloc.memorylocations[0].name
            if alloc.kind == "ExternalInput":
                if name != partition_name:
                    in_names.append(name)
            elif alloc.kind == "ExternalOutput":
                shape = tuple(alloc.tensor_shape)
                dtype = mybir.dt.np(alloc.dtype)
                out_avals.append(jax.core.ShapedArray(shape, dtype))
                out_names.append(name)
        self.in_names, self.out_names = in_names, out_names
        self.out_avals = out_avals
        n_params, n_outs = len(in_names), len(out_avals)
        all_in = list(in_names) + list(out_names)
        if partition_name is not None:
            all_in.append(partition_name)

        def _body(*args):
            operands = list(args)
            if partition_name is not None:
                operands.append(partition_id_tensor())
            return tuple(_bass_exec_p.bind(
                *operands, out_avals=tuple(out_avals), in_names=tuple(all_in),
                out_names=tuple(out_names), lowering_input_output_aliases=(),
                sim_require_finite=True, sim_require_nnan=True, nc=nc))

        devices = jax.devices()[:n_cores]
        self.mesh = Mesh(np.asarray(devices), ("core",))
        in_specs = (PartitionSpec("core"),) * (n_params + n_outs)
        out_specs = (PartitionSpec("core"),) * n_outs
        self.fn = jax.jit(
            shard_map(_body, mesh=self.mesh, in_specs=in_specs,
                      out_specs=out_specs, check_rep=False),
            keep_unused=True)
        self.sharding = NamedSharding(self.mesh, PartitionSpec("core"))

    def put_inputs(self, in_maps):
        n = self.n_cores
        per_core = [[np.asarray(m[k]) for k in self.in_names] for m in in_maps]
        self.dev_in = [
            jax.device_put(
                np.concatenate([per_core[c][i] for c in range(n)], axis=0),
                self.sharding)
            for i in range(len(self.in_names))
        ]
        for a in self.dev_in:
            a.block_until_ready()
        # dummy output operands (reused every run; kernel writes results into
        # the custom call's result buffers, not these)
        self.zs = [
            jax.device_put(
                np.zeros((n * a.shape[0],) + a.shape[1:], a.dtype),
                self.sharding)
            for a in self.out_avals
        ]
        for z in self.zs:
            z.block_until_ready()

    def run(self, fetch=True):
        n = self.n_cores
        outs = self.fn(*self.dev_in, *self.zs)
        for o in outs:
            o.block_until_ready()
        if not fetch:
            return None
        return [
            {name: np.asarray(outs[i]).reshape(n, *self.out_avals[i].shape)[c]
             for i, name in enumerate(self.out_names)}
            for c in range(n)
        ]


# ---------------------------------------------------------------------------
# Entry point
# ---------------------------------------------------------------------------

_LAST_RUNNER = None


def kernel(gcn_src1, gcn_dst1, gcn_src2, gcn_dst2,
           rg_src1, rg_dst1, rg_et1, rg_src2, rg_dst2, rg_et2,
           het_src1, het_dst1, het_src2, het_dst2,
           emb, gcn_W1, gcn_b1, gcn_W2, gcn_b2,
           rg_W1, rg_loop1, rg_b1, rg_W2, rg_loop2, rg_b2,
           het_W1, het_b1, het_W2, het_b2):
    emb = np.asarray(emb, np.float32)

    # hetero edge lists: concatenate the 4 relations with rel tags
    def het_edges(srcs, dsts):
        s = np.concatenate([np.asarray(srcs[r]).ravel() for r in range(R_HET)])
        d = np.concatenate([np.asarray(dsts[r]).ravel() for r in range(R_HET)])
        r = np.concatenate([np.full(np.asarray(srcs[r]).size, r, np.int64)
                            for r in range(R_HET)])
        return s, d, r

    hs1_, hd1_, hr1_ = het_edges(het_src1, het_dst1)
    hs2_, hd2_, hr2_ = het_edges(het_src2, het_dst2)

    plans = {
        "gcn1": pack_layer(gcn_src1, gcn_dst1, None, 1, ALIGN["gcn1"]),
        "gcn2": pack_layer(gcn_src2, gcn_dst2, None, 1, ALIGN["gcn2"]),
        "rg1": pack_layer(rg_src1, rg_dst1, rg_et1, R_RG, ALIGN["rg1"]),
        "rg2": pack_layer(rg_src2, rg_dst2, rg_et2, R_RG, ALIGN["rg2"]),
        "het1": pack_layer(hs1_, hd1_, hr1_, R_HET, ALIGN["het1"]),
        "het2": pack_layer(hs2_, hd2_, hr2_, R_HET, ALIGN["het2"]),
    }

    nc = build_program(plans)
    runner = _Runner(nc, NCORES)

    iota_np = np.broadcast_to(np.arange(D, dtype=np.float32), (D, D))
    shared = {
        "gcn_W1": np.asarray(gcn_W1).astype(TNP),
        "gcn_W2": np.asarray(gcn_W2).astype(TNP),
        "gcn_b1": np.asarray(gcn_b1, np.float32).reshape(D, 1),
        "gcn_b2": np.asarray(gcn_b2, np.float32).reshape(D, 1),
        "rg_W1": np.concatenate([np.asarray(rg_W1)[r] for r in range(R_RG)],
                                axis=1).astype(TNP),
        "rg_W2": np.concatenate([np.asarray(rg_W2)[r] for r in range(R_RG)],
                                axis=1).astype(TNP),
        "rg_loop1": np.asarray(rg_loop1).astype(TNP),
        "rg_loop2": np.asarray(rg_loop2).astype(TNP),
        "rg_b1": np.asarray(rg_b1, np.float32).reshape(D, 1),
        "rg_b2": np.asarray(rg_b2, np.float32).reshape(D, 1),
        "het_W1": np.concatenate([np.asarray(het_W1)[r] for r in range(R_HET)],
                                 axis=1).astype(TNP),
        "het_W2": np.concatenate([0.25 * np.asarray(het_W2)[r]
                                  for r in range(R_HET)], axis=1).astype(TNP),
        "het_b1": np.ascontiguousarray(np.asarray(het_b1, np.float32).T),
        "het_b2": np.ascontiguousarray(np.asarray(het_b2, np.float32).T),
        "iota": iota_np.astype(TNP),
        "ident_b": np.eye(D, dtype=TNP),
    }

    in_maps = []
    for c in range(NCORES):
        m = dict(shared)
        embT = np.zeros((D, NTILE * 128), np.float32)
        embT[:, :NLOC] = emb[c * NLOC:(c + 1) * NLOC, :].T
        m["embT_in"] = embT.astype(TNP)
        for lname in ("gcn1", "gcn2", "rg1", "rg2", "het1", "het2"):
            m[f"idx_{lname}"] = plans[lname]["idx"][c]
            m[f"dl_{lname}"] = plans[lname]["dl"][c].astype(TNP)
        in_maps.append(m)

    global _LAST_RUNNER
    _LAST_RUNNER = runner
    runner.put_inputs(in_maps)
    res = runner.run()

    hcf = np.concatenate([res[c]["hcfT"].T for c in range(NCORES)], axis=0)
    hc = np.concatenate([res[c]["hcT"].T for c in range(NCORES)], axis=0)
    hs = np.concatenate([res[c]["hsT"].T for c in range(NCORES)], axis=0)
    return (hcf, hc, hs)


# revision 13
# speedup vs baseline: 10.0483x; 5.1119x over previous
"""Trainium2 Bass kernel for CSNetModel GNN message passing (8 NeuronCores).

Strategy: shard destination nodes across the 8 cores (12500 each). Each layer's
segment_sum is computed with one-hot matmuls on the tensor engine over
host-sorted edge chunks; per-edge features are fetched with indirect DMA
gathers from replicated (AllGather'd) bf16 node tables. Feature transforms are
fused before aggregation (GCN: gather pre-transformed tables) or after
(RGCN/Hetero: per-relation PSUM banks + weight matmuls). All index arithmetic
is done on the host; the device program is identical across cores (SPMD), with
per-core edge data padded to a uniform chunk/segment schedule.

I/O layout note: through this runtime path, every ExternalInput/Output costs
per-exec marshalling proportional to its number of DRAM rows (dim0), so all
external tensors are shaped [128, *] (wide). Outputs are produced transposed
([feat, node]) and un-transposed on the host; emb arrives tile-transposed.
"""
import math
import numpy as np
import ml_dtypes

import jax
from jax.sharding import Mesh, PartitionSpec, NamedSharding
from jax.experimental.shard_map import shard_map

import concourse.bass as bass
import concourse.bacc as bacc
import concourse.tile as tile
import concourse.mybir as mybir
from concourse.bass2jax import _bass_exec_p, install_neuronx_cc_hook, partition_id_tensor

F32 = mybir.dt.float32
BF16 = mybir.dt.bfloat16
I32 = mybir.dt.int32

NCORES = 8
N = 100000
NLOC = N // NCORES          # 12500
D = 128
NTILE = (NLOC + 127) // 128  # 98
LAST_W = NLOC - (NTILE - 1) * 128  # 84
R_HET = 4
R_RG = 8

TDT = BF16                   # table / matmul dtype
TNP = ml_dtypes.bfloat16

ALIGN = {"gcn1": False, "gcn2": False, "rg1": False, "rg2": False,
         "het1": False, "het2": False}


def _tw(t):
    return 128 if t < NTILE - 1 else LAST_W


# ---------------------------------------------------------------------------
# Host-side edge packing
# ---------------------------------------------------------------------------

def pack_layer(src, dst, rel, R, align):
    """Build SPMD-uniform chunk/segment schedule for one layer-graph.

    src, dst: int arrays [E] (global node ids); rel: int array [E] or None.
    Returns dict with nchunk, nseg, groups (ordered list), and per-core
    idx_mat [128, nchunk] int32 / dl_mat [128, nseg] float32.
    """
    src = np.asarray(src).astype(np.int64)
    dst = np.asarray(dst).astype(np.int64)
    rel = np.zeros_like(src) if rel is None else np.asarray(rel).astype(np.int64)
    core = dst // NLOC
    dl = dst % NLOC
    tl = dl // 128
    dloc = dl % 128
    g = tl * R + rel
    NG = NTILE * R

    counts = np.zeros((NCORES, NG), np.int64)
    percore = []
    for c in range(NCORES):
        m = core == c
        gc = g[m]
        order = np.argsort(gc, kind="stable")
        gc = gc[order]
        percore.append((gc, src[m][order], dloc[m][order]))
        counts[c] = np.bincount(gc, minlength=NG)
    NE = counts.max(axis=0)

    if align:
        sizes = ((NE + 127) // 128) * 128
    else:
        sizes = NE.copy()
    off = np.zeros(NG + 1, np.int64)
    np.cumsum(sizes, out=off[1:])
    total = int(off[-1])
    nchunk = (total + 127) // 128
    tot_pad = nchunk * 128

    groups = []
    nseg = 0
    for gi in range(NG):
        ne = int(NE[gi])
        if ne == 0:
            continue
        lo, hi = int(off[gi]), int(off[gi]) + ne
        segs = []
        for k in range(lo // 128, (hi - 1) // 128 + 1):
            segs.append((k, nseg))
            nseg += 1
        groups.append({"t": gi // R, "r": gi % R, "lo": lo, "hi": hi, "segs": segs})

    group_of = {(grp["t"], grp["r"]): grp for grp in groups}

    idx_mats, dl_mats = [], []
    starts = off[:-1]
    for c in range(NCORES):
        gc, srcs, dlocs = percore[c]
        first_occ = np.searchsorted(gc, np.arange(NG))
        pos = starts[gc] + (np.arange(len(gc)) - first_occ[gc])
        idx_flat = np.zeros(tot_pad, np.int32)
        idx_flat[pos] = srcs
        dl_flat = np.full(tot_pad, -1.0, np.float32)
        dl_flat[pos] = dlocs
        idx_mats.append(np.ascontiguousarray(idx_flat.reshape(nchunk, 128).T))
        dl_mat = np.full((128, max(nseg, 1)), -1.0, np.float32)
        for grp in groups:
            for (k, col) in grp["segs"]:
                s = max(grp["lo"], k * 128)
                e = min(grp["hi"], (k + 1) * 128)
                colv = np.full(128, -1.0, np.float32)
                colv[s - k * 128:e - k * 128] = dl_flat[s:e]
                dl_mat[:, col] = colv
        dl_mats.append(dl_mat)

    return {"nchunk": nchunk, "nseg": max(nseg, 1), "groups": groups,
            "group_of": group_of, "idx": idx_mats, "dl": dl_mats}


# ---------------------------------------------------------------------------
# Device program
# ---------------------------------------------------------------------------

def build_program(plans, stages=("prep", "ag01", "l1", "ag234", "l2")):
    stages = set(stages)
    noag = "noag" in stages
    nc = bacc.Bacc("TRN2", target_bir_lowering=False, debug=False,
                   num_devices=NCORES)

    # --- external inputs (per core) ---
    ext = {}

    def din(name, shape, dt):
        ext[name] = nc.dram_tensor(name, list(shape), dt, kind="ExternalInput")
        return ext[name]

    embT_in = din("embT_in", [D, NTILE * 128], TDT)  # tile-transposed emb slice
    gcn_W1 = din("gcn_W1", [D, D], TDT)
    gcn_W2 = din("gcn_W2", [D, D], TDT)
    gcn_b1 = din("gcn_b1", [D, 1], F32)
    gcn_b2 = din("gcn_b2", [D, 1], F32)
    rg_W1 = din("rg_W1", [D, R_RG * D], TDT)
    rg_W2 = din("rg_W2", [D, R_RG * D], TDT)
    rg_loop1 = din("rg_loop1", [D, D], TDT)
    rg_loop2 = din("rg_loop2", [D, D], TDT)
    rg_b1 = din("rg_b1", [D, 1], F32)
    rg_b2 = din("rg_b2", [D, 1], F32)
    het_W1 = din("het_W1", [D, R_HET * D], TDT)
    het_W2 = din("het_W2", [D, R_HET * D], TDT)    # pre-scaled by 0.25 on host
    het_b1 = din("het_b1", [D, R_HET], F32)
    het_b2 = din("het_b2", [D, R_HET], F32)
    iota_in = din("iota", [D, D], TDT)
    ident_b = din("ident_b", [D, D], TDT)
    for lname in ("gcn1", "gcn2", "rg1", "rg2", "het1", "het2"):
        p = plans[lname]
        din(f"idx_{lname}", [128, p["nchunk"]], I32)
        din(f"dl_{lname}", [128, p["nseg"]], TDT)

    # single transposed output [feat, 3*node]: cols [0:NLOC)=hcf,
    # [NLOC:2NLOC)=hc, [2NLOC:3NLOC)=hs. One ExternalOutput only — each
    # additional output tensor costs ~67ms/exec through this runtime path.
    out_all = nc.dram_tensor("outT", [D, 3 * NLOC], F32, kind="ExternalOutput")
    OFF_HCF, OFF_HC, OFF_HS = 0, NLOC, 2 * NLOC

    if noag:
        ext_tabs = {}
        for nm in ("emb_full_in", "t1_full_in", "t2_full_in", "h1_full_in",
                   "hs1_full_in"):
            ext_tabs[nm] = nc.dram_tensor(nm, [N, D], TDT, kind="ExternalInput")

    Tanh = mybir.ActivationFunctionType.Tanh
    AG = "AllGather"
    RGROUPS = [list(range(NCORES))]

    with tile.TileContext(nc) as tc:
        with tc.tile_pool(name="consts", bufs=1) as cp, \
             tc.tile_pool(name="gat", bufs=28) as gp, \
             tc.tile_pool(name="oh", bufs=12) as ohp, \
             tc.tile_pool(name="work", bufs=6) as wp, \
             tc.tile_pool(name="psb", bufs=3, space="PSUM") as psb, \
             tc.tile_pool(name="pss", bufs=3, space="PSUM") as pss, \
             tc.tile_pool(name="ptr", bufs=2, space="PSUM") as ptr, \
             tc.tile_pool(name="dram", bufs=1, space="DRAM") as dp:

            # --- constant tiles ---
            def load_const(name, shape, dt):
                t = cp.tile(list(shape), dt, tag=name)
                nc.sync.dma_start(out=t[:], in_=ext[name][:])
                return t

            iota_t = load_const("iota", [D, D], TDT)
            identb_t = load_const("ident_b", [D, D], TDT)
            gW1_t = load_const("gcn_W1", [D, D], TDT)
            gW2_t = load_const("gcn_W2", [D, D], TDT)
            gb1_t = load_const("gcn_b1", [D, 1], F32)
            gb2_t = load_const("gcn_b2", [D, 1], F32)
            rW1_t = load_const("rg_W1", [D, R_RG * D], TDT)
            rW2_t = load_const("rg_W2", [D, R_RG * D], TDT)
            rL1_t = load_const("rg_loop1", [D, D], TDT)
            rL2_t = load_const("rg_loop2", [D, D], TDT)
            rb1_t = load_const("rg_b1", [D, 1], F32)
            rb2_t = load_const("rg_b2", [D, 1], F32)
            hW1_t = load_const("het_W1", [D, R_HET * D], TDT)
            hW2_t = load_const("het_W2", [D, R_HET * D], TDT)
            hb1_t = load_const("het_b1", [D, R_HET], F32)
            hb2_t = load_const("het_b2", [D, R_HET], F32)
            meta = {}
            for lname in ("gcn1", "gcn2", "rg1", "rg2", "het1", "het2"):
                p = plans[lname]
                meta[lname] = (
                    load_const(f"idx_{lname}", [128, p["nchunk"]], I32),
                    load_const(f"dl_{lname}", [128, p["nseg"]], TDT),
                )

            # --- internal DRAM ---
            emb_bounce = dp.tile([NLOC, D], TDT, tag="emb_b")
            emb_full = dp.tile([N, D], TDT, tag="emb_f", addr_space="Shared")
            t1_bounce = dp.tile([NLOC, D], TDT, tag="t1_b")
            t1_full = dp.tile([N, D], TDT, tag="t1_f", addr_space="Shared")
            t2_bounce = dp.tile([NLOC, D], TDT, tag="t2_b")
            t2_full = dp.tile([N, D], TDT, tag="t2_f", addr_space="Shared")
            h1_bounce = dp.tile([NLOC, D], TDT, tag="h1_b")
            h1_full = dp.tile([N, D], TDT, tag="h1_f", addr_space="Shared")
            hs1_bounce = dp.tile([NLOC, D], TDT, tag="hs1_b")
            hs1_full = dp.tile([N, D], TDT, tag="hs1_f", addr_space="Shared")
            h1T_dram = dp.tile([D, NLOC], TDT, tag="h1T")

            # --- prep: emb_bounce rows from embT tiles, T1 = emb @ W1 ---
            for t in range(NTILE):
                w = _tw(t)
                eT = wp.tile([128, D], TDT, tag="eT")
                nc.sync.dma_start(out=eT[:], in_=embT_in[:, t * 128:t * 128 + 128])
                trp = ptr.tile([128, D], TDT, tag="ptr")
                nc.tensor.transpose(out=trp[:], in_=eT[:], identity=identb_t[:])
                e_sb = wp.tile([128, D], TDT, tag="embt")
                nc.vector.tensor_copy(out=e_sb[:], in_=trp[:])
                nc.sync.dma_start(out=emb_bounce[t * 128:t * 128 + w, :],
                                  in_=e_sb[:w, :])
                t1p = pss.tile([128, D], F32, tag="pss")
                nc.tensor.matmul(out=t1p[:], lhsT=eT[:], rhs=gW1_t[:],
                                 start=True, stop=True)
                t1sb = wp.tile([128, D], TDT, tag="t1sb")
                nc.vector.tensor_copy(out=t1sb[:], in_=t1p[:])
                nc.sync.dma_start(out=t1_bounce[t * 128:t * 128 + w, :],
                                  in_=t1sb[:w, :])

            if "ag01" in stages:
                nc.gpsimd.collective_compute(
                    AG, mybir.AluOpType.bypass, replica_groups=RGROUPS,
                    ins=[emb_bounce.opt()], outs=[emb_full.opt()])
                nc.gpsimd.collective_compute(
                    AG, mybir.AluOpType.bypass, replica_groups=RGROUPS,
                    ins=[t1_bounce.opt()], outs=[t1_full.opt()])

            # --- shared layer machinery ---
            def gather_fn(lname, table):
                idx_t, _ = meta[lname]
                cache = {}

                def gather(k):
                    if k not in cache:
                        gt = gp.tile([128, D], TDT, tag="gat")
                        nc.gpsimd.indirect_dma_start(
                            out=gt[:], out_offset=None, in_=table[:],
                            in_offset=bass.IndirectOffsetOnAxis(
                                ap=idx_t[:, k:k + 1], axis=0))
                        cache[k] = gt
                    return cache[k]
                return gather

            def onehot(lname, col):
                _, dl_t = meta[lname]
                oh = ohp.tile([128, D], TDT, tag="oh")
                nc.vector.tensor_tensor(
                    out=oh[:], in0=dl_t[:, col:col + 1].to_broadcast([128, D]),
                    in1=iota_t[:], op=mybir.AluOpType.is_equal)
                return oh

            def accum_group(lname, gather, grp, bank, bcol, transposed):
                """Accumulate one (tile, rel) group into bank[:, bcol:bcol+128].

                transposed=True -> out[f, d] (lhsT=msgs, rhs=onehot)
                transposed=False -> out[d, f] (lhsT=onehot, rhs=msgs)
                """
                segs = grp["segs"]
                for si, (k, col) in enumerate(segs):
                    gt = gather(k)
                    oh = onehot(lname, col)
                    lhsT, rhs = (gt, oh) if transposed else (oh, gt)
                    nc.tensor.matmul(out=bank[:, bcol:bcol + 128],
                                     lhsT=lhsT[:], rhs=rhs[:],
                                     start=(si == 0), stop=(si == len(segs) - 1))

            # =========== GCN layer 1 (gathers T1; aggT supertiles) ==========
            def emit_gcn1():
                lname = "gcn1"
                plan = plans[lname]
                gather = gather_fn(lname, t1_full)
                for st in range((NTILE + 3) // 4):
                    tls = list(range(st * 4, min(st * 4 + 4, NTILE)))
                    bank = psb.tile([128, 512], F32, tag="psb")
                    for j, t in enumerate(tls):
                        grp = plan["group_of"].get((t, 0))
                        if grp is None:
                            nc.vector.memset(bank[:, j * 128:(j + 1) * 128], 0.0)
                            continue
                        accum_group(lname, gather, grp, bank, j * 128, True)
                    w = 128 * len(tls)
                    h1T = wp.tile([128, 512], TDT, tag="h1Tst")
                    nc.scalar.activation(h1T[:, :w], bank[:, :w], Tanh,
                                         bias=gb1_t[:], scale=1.0)
                    for j, t in enumerate(tls):
                        tp = pss.tile([128, D], F32, tag="pss")
                        nc.tensor.matmul(out=tp[:],
                                         lhsT=h1T[:, j * 128:(j + 1) * 128],
                                         rhs=gW2_t[:], start=True, stop=True)
                        tsb = wp.tile([128, D], TDT, tag="t2sb")
                        nc.vector.tensor_copy(out=tsb[:], in_=tp[:])
                        nc.sync.dma_start(
                            out=t2_bounce[t * 128:t * 128 + _tw(t), :],
                            in_=tsb[:_tw(t), :])

            # ====== GCN layer 2 (gathers T2; agg transposed, direct out) ====
            def emit_gcn2():
                lname = "gcn2"
                plan = plans[lname]
                gather = gather_fn(lname, t2_full)
                for t in range(NTILE):
                    w = _tw(t)
                    grp = plan["group_of"].get((t, 0))
                    pt = pss.tile([128, D], F32, tag="pss")
                    if grp is None:
                        nc.vector.memset(pt[:], 0.0)
                    else:
                        accum_group(lname, gather, grp, pt, 0, True)
                    ot = wp.tile([128, D], F32, tag="g2out")
                    nc.scalar.activation(ot[:], pt[:], Tanh, bias=gb2_t[:],
                                         scale=1.0)
                    nc.sync.dma_start(
                        out=out_all[:, OFF_HCF + t * 128:OFF_HCF + t * 128 + w],
                        in_=ot[:, :w])

            # =========== RGCN layer (B banks per rel + transforms) ==========
            def emit_rg(lname, table, xT_src, W_t, loop_t, b_t, first):
                plan = plans[lname]
                gather = gather_fn(lname, table)
                for t in range(NTILE):
                    w = _tw(t)
                    quads = []
                    for qi in range(2):
                        q = psb.tile([128, 512], F32, tag="psb")
                        quads.append(q)
                    for r in range(R_RG):
                        grp = plan["group_of"].get((t, r))
                        q, qc = quads[r // 4], (r % 4) * 128
                        if grp is None:
                            nc.vector.memset(q[:, qc:qc + 128], 0.0)
                        else:
                            accum_group(lname, gather, grp, q, qc, True)
                    stages_sb = []
                    for qi in range(2):
                        s = wp.tile([128, 512], TDT, tag="stage")
                        nc.vector.tensor_copy(out=s[:], in_=quads[qi][:])
                        stages_sb.append(s)
                    xT_t = wp.tile([128, D], TDT, tag="xTt")
                    nc.sync.dma_start(out=xT_t[:, :w],
                                      in_=xT_src[:, t * 128:t * 128 + w])
                    ot = pss.tile([128, D], F32, tag="pss")
                    nc.tensor.matmul(out=ot[:], lhsT=loop_t[:], rhs=xT_t[:],
                                     start=True, stop=False)
                    for r in range(R_RG):
                        nc.tensor.matmul(
                            out=ot[:], lhsT=W_t[:, r * 128:(r + 1) * 128],
                            rhs=stages_sb[r // 4][:, (r % 4) * 128:(r % 4 + 1) * 128],
                            start=False, stop=(r == R_RG - 1))
                    if first:
                        hT = wp.tile([128, D], TDT, tag="hTb")
                        nc.scalar.activation(hT[:], ot[:], Tanh, bias=b_t[:],
                                             scale=1.0)
                        nc.sync.dma_start(
                            out=h1T_dram[:, t * 128:t * 128 + w], in_=hT[:, :w])
                        trp = ptr.tile([128, D], TDT, tag="ptr")
                        nc.tensor.transpose(out=trp[:], in_=hT[:],
                                            identity=identb_t[:])
                        hsb = wp.tile([128, D], TDT, tag="hsbb")
                        nc.vector.tensor_copy(out=hsb[:], in_=trp[:])
                        nc.sync.dma_start(out=h1_bounce[t * 128:t * 128 + w, :],
                                          in_=hsb[:w, :])
                    else:
                        hTf = wp.tile([128, D], F32, tag="hTf")
                        nc.scalar.activation(hTf[:], ot[:], Tanh, bias=b_t[:],
                                             scale=1.0)
                        nc.sync.dma_start(
                            out=out_all[:, OFF_HC + t * 128:OFF_HC + t * 128 + w],
                            in_=hTf[:, :w])

            # =========== Hetero layer (4 rels, mean of tanh) ================
            def emit_het(lname, table, W_t, b_t, first):
                plan = plans[lname]
                gather = gather_fn(lname, table)
                for t in range(NTILE):
                    w = _tw(t)
                    quad = psb.tile([128, 512], F32, tag="psb")
                    for r in range(R_HET):
                        grp = plan["group_of"].get((t, r))
                        if grp is None:
                            nc.vector.memset(quad[:, r * 128:(r + 1) * 128], 0.0)
                        else:
                            accum_group(lname, gather, grp, quad, r * 128, True)
                    stage = wp.tile([128, 512], TDT, tag="stage")
                    nc.vector.tensor_copy(out=stage[:], in_=quad[:])
                    acc = wp.tile([128, D], F32, tag="hacc")
                    for r in range(R_HET):
                        otr = pss.tile([128, D], F32, tag="pss")
                        nc.tensor.matmul(
                            out=otr[:], lhsT=W_t[:, r * 128:(r + 1) * 128],
                            rhs=stage[:, r * 128:(r + 1) * 128],
                            start=True, stop=True)
                        if r == 0:
                            nc.scalar.activation(acc[:], otr[:], Tanh,
                                                 bias=b_t[:, 0:1], scale=1.0)
                        else:
                            tmp = wp.tile([128, D], F32, tag="htmp")
                            nc.scalar.activation(tmp[:], otr[:], Tanh,
                                                 bias=b_t[:, r:r + 1], scale=1.0)
                            nc.vector.tensor_add(out=acc[:], in0=acc[:],
                                                 in1=tmp[:])
                    if first:
                        # layer-1 mean (0.25) is folded into het_W2 on host
                        hsT = wp.tile([128, D], TDT, tag="hTb")
                        nc.vector.tensor_copy(out=hsT[:], in_=acc[:])
                        trp = ptr.tile([128, D], TDT, tag="ptr")
                        nc.tensor.transpose(out=trp[:], in_=hsT[:],
                                            identity=identb_t[:])
                        hsb = wp.tile([128, D], TDT, tag="hsbb")
                        nc.vector.tensor_copy(out=hsb[:], in_=trp[:])
                        nc.sync.dma_start(out=hs1_bounce[t * 128:t * 128 + w, :],
                                          in_=hsb[:w, :])
                    else:
                        hsT = wp.tile([128, D], F32, tag="hTf")
                        nc.vector.tensor_scalar_mul(hsT[:], acc[:], 0.25)
                        nc.sync.dma_start(
                            out=out_all[:, OFF_HS + t * 128:OFF_HS + t * 128 + w],
                            in_=hsT[:, :w])

            # --- emit layers ---
            emb_tab, t1_tab, t2_tab = emb_full, t1_full, t2_full
            h1_tab, hs1_tab = h1_full, hs1_full
            if noag:
                emb_tab = ext_tabs["emb_full_in"]
                t1_tab = ext_tabs["t1_full_in"]
                t2_tab = ext_tabs["t2_full_in"]
                h1_tab = ext_tabs["h1_full_in"]
                hs1_tab = ext_tabs["hs1_full_in"]
            if "l1" in stages or "l1rg" in stages:
                emit_rg("rg1", emb_tab, embT_in, rW1_t, rL1_t, rb1_t, True)
            if "l1" in stages or "l1het" in stages:
                emit_het("het1", emb_tab, hW1_t, hb1_t, True)
            if "l1" in stages or "l1gcn" in stages:
                emit_gcn1()

            if "ag234" in stages:
                nc.gpsimd.collective_compute(
                    AG, mybir.AluOpType.bypass, replica_groups=RGROUPS,
                    ins=[h1_bounce.opt()], outs=[h1_full.opt()])
                nc.gpsimd.collective_compute(
                    AG, mybir.AluOpType.bypass, replica_groups=RGROUPS,
                    ins=[hs1_bounce.opt()], outs=[hs1_full.opt()])
                nc.gpsimd.collective_compute(
                    AG, mybir.AluOpType.bypass, replica_groups=RGROUPS,
                    ins=[t2_bounce.opt()], outs=[t2_full.opt()])

            if "l2" in stages or "l2rg" in stages:
                emit_rg("rg2", h1_tab, h1T_dram, rW2_t, rL2_t, rb2_t, False)
            if "l2" in stages or "l2het" in stages:
                emit_het("het2", hs1_tab, hW2_t, hb2_t, False)
            if "l2" in stages or "l2gcn" in stages:
                emit_gcn2()

    nc.compile()
    return nc


# ---------------------------------------------------------------------------
# Runner (PJRT via axon)
# ---------------------------------------------------------------------------

class _Runner:
    def __init__(self, nc, n_cores):
        install_neuronx_cc_hook()
        self.n_cores = n_cores
        partition_name = (nc.partition_id_tensor.name
                          if nc.partition_id_tensor else None)
        in_names, out_names, out_avals = [], [], []
        for alloc in nc.m.functions[0].allocations:
            if not isinstance(alloc, mybir.MemoryLocationSet):
                continue
            name = alloc.memorylocations[0].name
            if alloc.kind == "ExternalInput":
                if name != partition_name:
                    in_names.append(name)
            elif alloc.kind == "ExternalOutput":
                shape = tuple(alloc.tensor_shape)
                dtype = mybir.dt.np(alloc.dtype)
                out_avals.append(jax.core.ShapedArray(shape, dtype))
                out_names.append(name)
        self.in_names, self.out_names = in_names, out_names
        self.out_avals = out_avals
        n_params, n_outs = len(in_names), len(out_avals)
        all_in = list(in_names) + list(out_names)
        if partition_name is not None:
            all_in.append(partition_name)

        def _body(*args):
            operands = list(args)
            if partition_name is not None:
                operands.append(partition_id_tensor())
            return tuple(_bass_exec_p.bind(
                *operands, out_avals=tuple(out_avals), in_names=tuple(all_in),
                out_names=tuple(out_names), lowering_input_output_aliases=(),
                sim_require_finite=True, sim_require_nnan=True, nc=nc))

        devices = jax.devices()[:n_cores]
        self.mesh = Mesh(np.asarray(devices), ("core",))
        in_specs = (PartitionSpec("core"),) * (n_params + n_outs)
        out_specs = (PartitionSpec("core"),) * n_outs
        self.fn = jax.jit(
            shard_map(_body, mesh=self.mesh, in_specs=in_specs,
                      out_specs=out_specs, check_rep=False),
            keep_unused=True)
        self.sharding = NamedSharding(self.mesh, PartitionSpec("core"))

    def put_inputs(self, in_maps):
        n = self.n_cores
        per_core = [[np.asarray(m[k]) for k in self.in_names] for m in in_maps]
        self.dev_in = [
            jax.device_put(
                np.concatenate([per_core[c][i] for c in range(n)], axis=0),
                self.sharding)
            for i in range(len(self.in_names))
        ]
        for a in self.dev_in:
            a.block_until_ready()
        # dummy output operands (reused every run; kernel writes results into
        # the custom call's result buffers, not these)
        self.zs = [
            jax.device_put(
                np.zeros((n * a.shape[0],) + a.shape[1:], a.dtype),
                self.sharding)
            for a in self.out_avals
        ]
        for z in self.zs:
            z.block_until_ready()

    def run(self, fetch=True):
        n = self.n_cores
        outs = self.fn(*self.dev_in, *self.zs)
        for o in outs:
            o.block_until_ready()
        if not fetch:
            return None
        return [
            {name: np.asarray(outs[i]).reshape(n, *self.out_avals[i].shape)[c]
             for i, name in enumerate(self.out_names)}
            for c in range(n)
        ]


# ---------------------------------------------------------------------------
# Entry point
# ---------------------------------------------------------------------------

_LAST_RUNNER = None


def kernel(gcn_src1, gcn_dst1, gcn_src2, gcn_dst2,
           rg_src1, rg_dst1, rg_et1, rg_src2, rg_dst2, rg_et2,
           het_src1, het_dst1, het_src2, het_dst2,
           emb, gcn_W1, gcn_b1, gcn_W2, gcn_b2,
           rg_W1, rg_loop1, rg_b1, rg_W2, rg_loop2, rg_b2,
           het_W1, het_b1, het_W2, het_b2):
    emb = np.asarray(emb, np.float32)

    # hetero edge lists: concatenate the 4 relations with rel tags
    def het_edges(srcs, dsts):
        s = np.concatenate([np.asarray(srcs[r]).ravel() for r in range(R_HET)])
        d = np.concatenate([np.asarray(dsts[r]).ravel() for r in range(R_HET)])
        r = np.concatenate([np.full(np.asarray(srcs[r]).size, r, np.int64)
                            for r in range(R_HET)])
        return s, d, r

    hs1_, hd1_, hr1_ = het_edges(het_src1, het_dst1)
    hs2_, hd2_, hr2_ = het_edges(het_src2, het_dst2)

    plans = {
        "gcn1": pack_layer(gcn_src1, gcn_dst1, None, 1, ALIGN["gcn1"]),
        "gcn2": pack_layer(gcn_src2, gcn_dst2, None, 1, ALIGN["gcn2"]),
        "rg1": pack_layer(rg_src1, rg_dst1, rg_et1, R_RG, ALIGN["rg1"]),
        "rg2": pack_layer(rg_src2, rg_dst2, rg_et2, R_RG, ALIGN["rg2"]),
        "het1": pack_layer(hs1_, hd1_, hr1_, R_HET, ALIGN["het1"]),
        "het2": pack_layer(hs2_, hd2_, hr2_, R_HET, ALIGN["het2"]),
    }

    nc = build_program(plans)
    runner = _Runner(nc, NCORES)

    iota_np = np.broadcast_to(np.arange(D, dtype=np.float32), (D, D))
    shared = {
        "gcn_W1": np.asarray(gcn_W1).astype(TNP),
        "gcn_W2": np.asarray(gcn_W2).astype(TNP),
        "gcn_b1": np.asarray(gcn_b1, np.float32).reshape(D, 1),
        "gcn_b2": np.asarray(gcn_b2, np.float32).reshape(D, 1),
        "rg_W1": np.concatenate([np.asarray(rg_W1)[r] for r in range(R_RG)],
                                axis=1).astype(TNP),
        "rg_W2": np.concatenate([np.asarray(rg_W2)[r] for r in range(R_RG)],
                                axis=1).astype(TNP),
        "rg_loop1": np.asarray(rg_loop1).astype(TNP),
        "rg_loop2": np.asarray(rg_loop2).astype(TNP),
        "rg_b1": np.asarray(rg_b1, np.float32).reshape(D, 1),
        "rg_b2": np.asarray(rg_b2, np.float32).reshape(D, 1),
        "het_W1": np.concatenate([np.asarray(het_W1)[r] for r in range(R_HET)],
                                 axis=1).astype(TNP),
        "het_W2": np.concatenate([0.25 * np.asarray(het_W2)[r]
                                  for r in range(R_HET)], axis=1).astype(TNP),
        "het_b1": np.ascontiguousarray(np.asarray(het_b1, np.float32).T),
        "het_b2": np.ascontiguousarray(np.asarray(het_b2, np.float32).T),
        "iota": iota_np.astype(TNP),
        "ident_b": np.eye(D, dtype=TNP),
    }

    in_maps = []
    for c in range(NCORES):
        m = dict(shared)
        embT = np.zeros((D, NTILE * 128), np.float32)
        embT[:, :NLOC] = emb[c * NLOC:(c + 1) * NLOC, :].T
        m["embT_in"] = embT.astype(TNP)
        for lname in ("gcn1", "gcn2", "rg1", "rg2", "het1", "het2"):
            m[f"idx_{lname}"] = plans[lname]["idx"][c]
            m[f"dl_{lname}"] = plans[lname]["dl"][c].astype(TNP)
        in_maps.append(m)

    global _LAST_RUNNER
    _LAST_RUNNER = runner
    runner.put_inputs(in_maps)
    res = runner.run()

    hcf = np.concatenate([res[c]["outT"][:, 0:NLOC].T
                          for c in range(NCORES)], axis=0)
    hc = np.concatenate([res[c]["outT"][:, NLOC:2 * NLOC].T
                         for c in range(NCORES)], axis=0)
    hs = np.concatenate([res[c]["outT"][:, 2 * NLOC:3 * NLOC].T
                         for c in range(NCORES)], axis=0)
    return (hcf, hc, hs)
